# revision 16
# baseline (speedup 1.0000x reference)
"""Trainium2 Bass kernel for the gnn_message_passing problem.

Math reduction: the reference builds a [8192,8192] zero-diagonal adjacency
W_full from per-node Linear(8191,1) weights, forms state = [x | zeros] and
returns (state @ W_full.T + bias)[:, 7168:][:, ::-1].

Because state is zero outside its first 1024 columns, and only output nodes
7168..8191 are read, the whole computation collapses to

    out[b, k] = sum_c x[b, c] * weights[8191-k, c] + bias[8191-k]

i.e. a [32,1024] x [1024,1024]^T matmul + bias (for rows n >= 7168 and
cols c < 1024 we always have c < n, so W_full[n, c] == weights[n, c]).

Distribution: shard the 1024 output features row-wise across 8 cores
(128 each, tensor parallel); every core holds the replicated x. No
collectives — the host concatenates the 8 output slices.

Per-core kernel (raw bacc, hand-rolled semaphores — no TileContext, whose
drain/barrier/sem-clear tail costs ~2us): weights and x are cast to bf16 on
host (measured rel err ~2.7e-3 vs the f32 reference, well under the 2e-2
gate) and packed into TWO [128, 640] dram tensors, each carrying 4
contraction chunks of W plus the matching 4 chunks of x, so each HWDGE ring
(SP, ACT) moves one big DMA with 1.25KB-per-partition descriptors; the f32
bias trails ring A, gated by its own semaphore so it cannot delay the
weights. Eight PSUM-accumulated bf16 matmuls run as ring data lands; the
epilogue is a single DVE tensor_scalar add (PSUM + bias -> SBUF — no PE
bias-matmul in the accumulation tail and no activation-table load that
would stall the ACT HWDGE ring). SP issues the output DMA with its
mandatory completion increments but does NOT wait on them — the runtime's
end-of-execution epilogue drains the DMA queues, so the in-flight store
overlaps the fixed ~7us teardown sweep (validated by the --warm rerun in
test.py) — then resets the semaphores in one range clear so repeated NEFF
executions stay correct.
"""

import numpy as np
import ml_dtypes

import concourse.bacc as bacc
import concourse.mybir as mybir
from concourse.bass_utils import run_bass_kernel_spmd

NODES = 8192
IN_F = 1024
OUT_F = 1024
B = 32
N_CORES = 8
KPC = OUT_F // N_CORES   # output features per core: 128
NCHUNK = IN_F // 128     # contraction chunks: 8
HALF = NCHUNK // 2       # chunks per ring: 4
WCOL = HALF * KPC        # weight cols per ring tensor: 512
XCOL = HALF * B          # x cols per ring tensor: 128
RCOL = WCOL + XCOL       # ring tensor free dim: 640 (ring A: +1 bias col)

F32 = mybir.dt.float32
BF16 = mybir.dt.bfloat16

_NC = None
LAST_RESULT = None  # BassKernelResults of the most recent run (for profiling)


def _build_nc():
    nc = bacc.Bacc(None, target_bir_lowering=False)

    # Ring tensors, packed on host (bf16):
    #   inX[p, n*KPC + k'] = W_eff[core*KPC + k', (n + X*HALF)*128 + p]  n<HALF
    #   inX[p, WCOL + n*B + b] = x[b, (n + X*HALF)*128 + p]
    #   ina[p, RCOL] = bias[core*KPC + p]
    ina = nc.dram_tensor("ina", [128, RCOL], BF16, kind="ExternalInput")
    inb = nc.dram_tensor("inb", [128, RCOL], BF16, kind="ExternalInput")
    bi = nc.dram_tensor("bi", [KPC, 1], F32, kind="ExternalInput")
    out = nc.dram_tensor("out", [KPC, B], F32, kind="ExternalOutput")

    a_t = nc.alloc_sbuf_tensor("a_t", [128, RCOL], BF16)
    b_t = nc.alloc_sbuf_tensor("b_t", [128, RCOL], BF16)
    bi_t = nc.alloc_sbuf_tensor("bi_t", [KPC, 1], F32)
    o_t = nc.alloc_sbuf_tensor("o_t", [KPC, B], F32)
    ps = nc.alloc_psum_tensor("ps", [KPC, B], F32)

    s_a = nc.alloc_semaphore("s_a")
    s_b = nc.alloc_semaphore("s_b")
    s_bias = nc.alloc_semaphore("s_bias")
    s_pe = nc.alloc_semaphore("s_pe")
    s_dve = nc.alloc_semaphore("s_dve")
    s_out = nc.alloc_semaphore("s_out")

    # One big DMA per HWDGE ring: W/x half A on SP, half B on ACT (ACT does
    # nothing else — no act-table load ahead on its ring). The f32 bias
    # trails ring A: its 128 thin descriptors run after inA's fat ones, so
    # they cannot delay s_a, and only the DVE epilogue waits on s_bias.
    nc.sync.dma_start(a_t[:], ina[:]).then_inc(s_a, 16)
    nc.sync.dma_start(bi_t[:], bi[:]).then_inc(s_bias, 16)
    nc.scalar.dma_start(b_t[:], inb[:]).then_inc(s_b, 16)

    # PE: 8 PSUM-accumulated matmuls, half A then half B.
    nc.tensor.wait_ge(s_a, 16)
    for n in range(HALF):
        nc.tensor.matmul(
            ps[:],
            a_t[:, n * KPC : (n + 1) * KPC],            # lhsT [c=128, k'=128]
            a_t[:, WCOL + n * B : WCOL + (n + 1) * B],  # rhs  [c=128, b=32]
            start=(n == 0),
            stop=False,
        )
    nc.tensor.wait_ge(s_b, 16)
    for n in range(HALF):
        mm = nc.tensor.matmul(
            ps[:],
            b_t[:, n * KPC : (n + 1) * KPC],
            b_t[:, WCOL + n * B : WCOL + (n + 1) * B],
            start=False,
            stop=(n == HALF - 1),
        )
    mm.then_inc(s_pe, 1)

    # DVE: PSUM + per-partition bias -> SBUF, in two 64-row halves so the
    # two half-stores can be issued concurrently from SP and ACT.
    H = KPC // 2
    nc.vector.wait_ge(s_pe, 1)
    nc.vector.wait_ge(s_bias, 16)
    nc.vector.tensor_scalar_add(o_t[:H], ps[:H], bi_t[:H]).then_inc(s_dve, 1)
    nc.vector.tensor_scalar_add(o_t[H:], ps[H:], bi_t[H:]).then_inc(s_dve, 1)

    # SP stores half 0, ACT half 1, in parallel; each then resets its share
    # of the sems while the stores are in flight. No completion WAIT on the
    # stores (see module docstring); s_out still gets the mandatory
    # completion increments, but nothing ever waits on it, so clearing it
    # early (possibly racing the increments) is harmless. ACT clears s_dve:
    # its s_dve>=2 wait retires after SP's s_dve>=1 wait is satisfied, and
    # SP has been parked on that wait since its DMA issues finished.
    nc.sync.wait_ge(s_dve, 1)
    nc.sync.dma_start(out[:H], o_t[:H]).then_inc(s_out, 16)
    for s in (s_a, s_b, s_bias, s_pe):
        nc.sync.sem_clear(s)
    nc.scalar.wait_ge(s_dve, 2)
    nc.scalar.dma_start(out[H:], o_t[H:]).then_inc(s_out, 16)
    nc.scalar.sem_clear(s_dve)
    nc.scalar.sem_clear(s_out)

    nc.finalize()
    return nc


def kernel(x: np.ndarray, weights: np.ndarray, bias: np.ndarray) -> np.ndarray:
    global _NC, LAST_RESULT
    if _NC is None:
        _NC = _build_nc()

    x = np.asarray(x, dtype=np.float32)
    weights = np.asarray(weights, dtype=np.float32)
    bias = np.asarray(bias, dtype=np.float32)

    # Effective dense weight block and bias (see module docstring).
    w_eff = weights[NODES - OUT_F :, :IN_F][::-1]  # [1024 (k), 1024 (c)]
    b_eff = bias[NODES - OUT_F :][::-1]            # [1024]

    # Pack per-core ring operands. w_eff[(i,k'),(n,p)] -> wt[i][p, (n,k')]
    wt_all = w_eff.reshape(N_CORES, KPC, NCHUNK, 128).transpose(0, 3, 2, 1)
    wt_all = wt_all.reshape(N_CORES, 128, NCHUNK, KPC).astype(ml_dtypes.bfloat16)
    # x[b, (n,p)] -> xt[p, (n,b)], replicated
    xt = (
        x.reshape(B, NCHUNK, 128).transpose(2, 1, 0).astype(ml_dtypes.bfloat16)
    )  # [p, n, b]
    b_all = np.ascontiguousarray(b_eff.reshape(N_CORES, KPC, 1))

    in_maps = []
    for i in range(N_CORES):
        ina = np.concatenate(
            [wt_all[i, :, :HALF].reshape(128, WCOL), xt[:, :HALF].reshape(128, XCOL)],
            axis=1,
        )
        inb = np.concatenate(
            [wt_all[i, :, HALF:].reshape(128, WCOL), xt[:, HALF:].reshape(128, XCOL)],
            axis=1,
        )
        in_maps.append(
            {
                "ina": np.ascontiguousarray(ina),
                "inb": np.ascontiguousarray(inb),
                "bi": b_all[i],
            }
        )
    LAST_RESULT = run_bass_kernel_spmd(_NC, in_maps, list(range(N_CORES)))

    # Gather: core i returns out[k', b] for k = i*KPC + k'.
    out_t = np.concatenate([r["out"] for r in LAST_RESULT.results], axis=0)
    return np.ascontiguousarray(out_t.T)


# revision 17
# speedup vs baseline: 1.0901x; 1.0901x over previous
"""Trainium2 Bass kernel for the gnn_message_passing problem.

Math reduction: the reference builds a [8192,8192] zero-diagonal adjacency
W_full from per-node Linear(8191,1) weights, forms state = [x | zeros] and
returns (state @ W_full.T + bias)[:, 7168:][:, ::-1].

Because state is zero outside its first 1024 columns, and only output nodes
7168..8191 are read, the whole computation collapses to

    out[b, k] = sum_c x[b, c] * weights[8191-k, c] + bias[8191-k]

i.e. a [32,1024] x [1024,1024]^T matmul + bias (for rows n >= 7168 and
cols c < 1024 we always have c < n, so W_full[n, c] == weights[n, c]).

Distribution: shard the 1024 output features row-wise across 8 cores
(128 each, tensor parallel); every core holds the replicated x. No
collectives — the host concatenates the 8 output slices.

Per-core kernel (raw bacc, hand-rolled semaphores — no TileContext, whose
drain/barrier/sem-clear tail costs ~2us): weights and x are cast to bf16 on
host (measured rel err ~2.7e-3 vs the f32 reference, well under the 2e-2
gate) and packed into TWO [128, 640] dram tensors, each carrying 4
contraction chunks of W plus the matching 4 chunks of x, so each HWDGE ring
(SP, ACT) moves one big DMA with 1.25KB-per-partition descriptors; the f32
bias trails ring A, gated by its own semaphore so it cannot delay the
weights. Eight PSUM-accumulated bf16 matmuls run as ring data lands; the
epilogue is a single DVE tensor_scalar add (PSUM + bias -> SBUF — no PE
bias-matmul in the accumulation tail and no activation-table load that
would stall the ACT HWDGE ring). SP issues the output DMA with its
mandatory completion increments but does NOT wait on them — the runtime's
end-of-execution epilogue drains the DMA queues, so the in-flight store
overlaps the fixed ~7us teardown sweep (validated by the --warm rerun in
test.py) — then resets the semaphores in one range clear so repeated NEFF
executions stay correct.
"""

import numpy as np
import ml_dtypes

import concourse.bacc as bacc
import concourse.mybir as mybir
from concourse.bass_utils import run_bass_kernel_spmd

NODES = 8192
IN_F = 1024
OUT_F = 1024
B = 32
N_CORES = 8
KPC = OUT_F // N_CORES   # output features per core: 128
NCHUNK = IN_F // 128     # contraction chunks: 8
HALF = NCHUNK // 2       # chunks per ring: 4
WCOL = HALF * KPC        # weight cols per ring tensor: 512
XCOL = HALF * B          # x cols per ring tensor: 128
RCOL = WCOL + XCOL       # ring tensor free dim: 640 (ring A: +1 bias col)

F32 = mybir.dt.float32
BF16 = mybir.dt.bfloat16

_NC = None
LAST_RESULT = None  # BassKernelResults of the most recent run (for profiling)


def _build_nc():
    nc = bacc.Bacc(None, target_bir_lowering=False)

    # Ring tensors, packed on host (bf16):
    #   inX[p, n*KPC + k'] = W_eff[core*KPC + k', (n + X*HALF)*128 + p]  n<HALF
    #   inX[p, WCOL + n*B + b] = x[b, (n + X*HALF)*128 + p]
    #   ina[p, RCOL] = bias[core*KPC + p]
    ina = nc.dram_tensor("ina", [128, RCOL], BF16, kind="ExternalInput")
    inb = nc.dram_tensor("inb", [128, RCOL], BF16, kind="ExternalInput")
    bi = nc.dram_tensor("bi", [KPC, 1], F32, kind="ExternalInput")
    out = nc.dram_tensor("out", [KPC, B], F32, kind="ExternalOutput")

    a_t = nc.alloc_sbuf_tensor("a_t", [128, RCOL], BF16)
    b_t = nc.alloc_sbuf_tensor("b_t", [128, RCOL], BF16)
    bi_t = nc.alloc_sbuf_tensor("bi_t", [KPC, 1], F32)
    o_t = nc.alloc_sbuf_tensor("o_t", [KPC, B], F32)
    ps = nc.alloc_psum_tensor("ps", [KPC, B], F32)

    s_a = nc.alloc_semaphore("s_a")
    s_b = nc.alloc_semaphore("s_b")
    s_bias = nc.alloc_semaphore("s_bias")
    s_pe = nc.alloc_semaphore("s_pe")
    s_dve = nc.alloc_semaphore("s_dve")
    s_out = nc.alloc_semaphore("s_out")
    sems = (s_a, s_b, s_bias, s_pe, s_dve, s_out)

    # One big DMA per HWDGE ring: W/x half A on SP, half B on ACT (ACT does
    # nothing else — no act-table load ahead on its ring). The f32 bias
    # trails ring A: its 128 thin descriptors run after inA's fat ones, so
    # they cannot delay s_a, and only the DVE epilogue waits on s_bias.
    nc.sync.dma_start(a_t[:], ina[:]).then_inc(s_a, 16)
    nc.sync.dma_start(bi_t[:], bi[:]).then_inc(s_bias, 16)
    nc.scalar.dma_start(b_t[:], inb[:]).then_inc(s_b, 16)

    # PE: 8 PSUM-accumulated matmuls, half A then half B.
    nc.tensor.wait_ge(s_a, 16)
    for n in range(HALF):
        nc.tensor.matmul(
            ps[:],
            a_t[:, n * KPC : (n + 1) * KPC],            # lhsT [c=128, k'=128]
            a_t[:, WCOL + n * B : WCOL + (n + 1) * B],  # rhs  [c=128, b=32]
            start=(n == 0),
            stop=False,
        )
    nc.tensor.wait_ge(s_b, 16)
    for n in range(HALF):
        mm = nc.tensor.matmul(
            ps[:],
            b_t[:, n * KPC : (n + 1) * KPC],
            b_t[:, WCOL + n * B : WCOL + (n + 1) * B],
            start=False,
            stop=(n == HALF - 1),
        )
    mm.then_inc(s_pe, 1)

    # DVE: PSUM + per-partition bias -> SBUF in one op.
    nc.vector.wait_ge(s_pe, 1)
    nc.vector.wait_ge(s_bias, 16)
    nc.vector.tensor_scalar_add(o_t[:], ps[:], bi_t[:]).then_inc(s_dve, 1)

    # SP: store, then reset the sems while the store is in flight. No
    # completion WAIT on the store (see module docstring); s_out still gets
    # the mandatory completion increments, but nothing ever waits on it, so
    # clearing it early (possibly racing the increments) is harmless.
    nc.sync.wait_ge(s_dve, 1)
    nc.sync.dma_start(out[:], o_t[:]).then_inc(s_out, 16)
    nums = sorted(s.num for s in sems)
    if nums == list(range(nums[0], nums[0] + len(nums))):
        nc.sync.sem_clear(range(nums[0], nums[-1] + 1))
    else:  # non-contiguous allocation — clear individually
        for s in sems:
            nc.sync.sem_clear(s)

    nc.finalize()
    return nc


def kernel(x: np.ndarray, weights: np.ndarray, bias: np.ndarray) -> np.ndarray:
    global _NC, LAST_RESULT
    if _NC is None:
        _NC = _build_nc()

    x = np.asarray(x, dtype=np.float32)
    weights = np.asarray(weights, dtype=np.float32)
    bias = np.asarray(bias, dtype=np.float32)

    # Effective dense weight block and bias (see module docstring).
    w_eff = weights[NODES - OUT_F :, :IN_F][::-1]  # [1024 (k), 1024 (c)]
    b_eff = bias[NODES - OUT_F :][::-1]            # [1024]

    # Pack per-core ring operands. w_eff[(i,k'),(n,p)] -> wt[i][p, (n,k')]
    wt_all = w_eff.reshape(N_CORES, KPC, NCHUNK, 128).transpose(0, 3, 2, 1)
    wt_all = wt_all.reshape(N_CORES, 128, NCHUNK, KPC).astype(ml_dtypes.bfloat16)
    # x[b, (n,p)] -> xt[p, (n,b)], replicated
    xt = (
        x.reshape(B, NCHUNK, 128).transpose(2, 1, 0).astype(ml_dtypes.bfloat16)
    )  # [p, n, b]
    b_all = np.ascontiguousarray(b_eff.reshape(N_CORES, KPC, 1))

    in_maps = []
    for i in range(N_CORES):
        ina = np.concatenate(
            [wt_all[i, :, :HALF].reshape(128, WCOL), xt[:, :HALF].reshape(128, XCOL)],
            axis=1,
        )
        inb = np.concatenate(
            [wt_all[i, :, HALF:].reshape(128, WCOL), xt[:, HALF:].reshape(128, XCOL)],
            axis=1,
        )
        in_maps.append(
            {
                "ina": np.ascontiguousarray(ina),
                "inb": np.ascontiguousarray(inb),
                "bi": b_all[i],
            }
        )
    LAST_RESULT = run_bass_kernel_spmd(_NC, in_maps, list(range(N_CORES)))

    # Gather: core i returns out[k', b] for k = i*KPC + k'.
    out_t = np.concatenate([r["out"] for r in LAST_RESULT.results], axis=0)
    return np.ascontiguousarray(out_t.T)


# revision 21
# speedup vs baseline: 1.0997x; 1.0089x over previous
"""Trainium2 Bass kernel for the gnn_message_passing problem.

Math reduction: the reference builds a [8192,8192] zero-diagonal adjacency
W_full from per-node Linear(8191,1) weights, forms state = [x | zeros] and
returns (state @ W_full.T + bias)[:, 7168:][:, ::-1].

Because state is zero outside its first 1024 columns, and only output nodes
7168..8191 are read, the whole computation collapses to

    out[b, k] = sum_c x[b, c] * weights[8191-k, c] + bias[8191-k]

i.e. a [32,1024] x [1024,1024]^T matmul + bias (for rows n >= 7168 and
cols c < 1024 we always have c < n, so W_full[n, c] == weights[n, c]).

Distribution: shard the 1024 output features row-wise across 8 cores
(128 each, tensor parallel); every core holds the replicated x. No
collectives — the host concatenates the 8 output slices.

Per-core kernel (raw bacc, hand-rolled semaphores — no TileContext, whose
drain/barrier/sem-clear tail costs ~2us): weights and x are cast to bf16 on
host (measured rel err ~2.7e-3 vs the f32 reference, well under the 2e-2
gate) and packed into TWO [128, 640] dram tensors, each carrying 4
contraction chunks of W plus the matching 4 chunks of x, so each HWDGE ring
(SP, ACT) moves one big DMA with 1.25KB-per-partition descriptors; the f32
bias trails ring A, gated by its own semaphore so it cannot delay the
weights. Eight PSUM-accumulated bf16 matmuls run as ring data lands; the
epilogue is a single DVE tensor_scalar add (PSUM + bias -> SBUF — no PE
bias-matmul in the accumulation tail and no activation-table load that
would stall the ACT HWDGE ring). SP issues the output DMA with its
mandatory completion increments but does NOT wait on them — the runtime's
end-of-execution epilogue drains the DMA queues, so the in-flight store
overlaps the fixed ~7us teardown sweep (validated by the --warm rerun in
test.py) — then resets the semaphores in one range clear so repeated NEFF
executions stay correct.
"""

import numpy as np
import ml_dtypes

import concourse.bacc as bacc
import concourse.mybir as mybir
from concourse.bass_utils import run_bass_kernel_spmd

NODES = 8192
IN_F = 1024
OUT_F = 1024
B = 32
N_CORES = 8
KPC = OUT_F // N_CORES   # output features per core: 128
NCHUNK = IN_F // 128     # contraction chunks: 8
# Ring A (SP HWDGE) streams ~115 GB/s but starts ~0.5us before ring B
# (ACT HWDGE, ~199 GB/s) — 3 chunks on A and 5 on B makes both streams
# finish together (measured: equal-split left ring A ~0.4us behind).
NA = 3                   # contraction chunks on ring A
NB = NCHUNK - NA         # contraction chunks on ring B: 5
CPB = KPC + B            # cols per chunk (weights + x block): 160

F32 = mybir.dt.float32
BF16 = mybir.dt.bfloat16

_NC = None
LAST_RESULT = None  # BassKernelResults of the most recent run (for profiling)


def _build_nc():
    nc = bacc.Bacc(None, target_bir_lowering=False)

    # Ring tensors, packed on host (bf16); ring X carries chunks [off, off+S):
    #   inX[p, n*KPC + k'] = W_eff[core*KPC + k', (n + off)*128 + p]  n<S
    #   inX[p, S*KPC + n*B + b] = x[b, (n + off)*128 + p]
    ina = nc.dram_tensor("ina", [128, NA * CPB], BF16, kind="ExternalInput")
    inb = nc.dram_tensor("inb", [128, NB * CPB], BF16, kind="ExternalInput")
    bi = nc.dram_tensor("bi", [KPC, 1], F32, kind="ExternalInput")
    out = nc.dram_tensor("out", [KPC, B], F32, kind="ExternalOutput")

    a_t = nc.alloc_sbuf_tensor("a_t", [128, NA * CPB], BF16)
    b_t = nc.alloc_sbuf_tensor("b_t", [128, NB * CPB], BF16)
    bi_t = nc.alloc_sbuf_tensor("bi_t", [KPC, 1], F32)
    o_t = nc.alloc_sbuf_tensor("o_t", [KPC, B], F32)
    ps = nc.alloc_psum_tensor("ps", [KPC, B], F32)

    s_a = nc.alloc_semaphore("s_a")
    s_b = nc.alloc_semaphore("s_b")
    s_bias = nc.alloc_semaphore("s_bias")
    s_pe = nc.alloc_semaphore("s_pe")
    s_dve = nc.alloc_semaphore("s_dve")
    s_out = nc.alloc_semaphore("s_out")
    sems = (s_a, s_b, s_bias, s_pe, s_dve, s_out)

    # One big DMA per HWDGE ring: W/x half A on SP, half B on ACT (ACT does
    # nothing else — no act-table load ahead on its ring). The f32 bias
    # trails ring A: its 128 thin descriptors run after inA's fat ones, so
    # they cannot delay s_a, and only the DVE epilogue waits on s_bias.
    nc.sync.dma_start(a_t[:], ina[:]).then_inc(s_a, 16)
    nc.sync.dma_start(bi_t[:], bi[:]).then_inc(s_bias, 16)
    nc.scalar.dma_start(b_t[:], inb[:]).then_inc(s_b, 16)

    # PE: 8 PSUM-accumulated matmuls, ring A's chunks then ring B's.
    wa, wb = NA * KPC, NB * KPC
    nc.tensor.wait_ge(s_a, 16)
    for n in range(NA):
        nc.tensor.matmul(
            ps[:],
            a_t[:, n * KPC : (n + 1) * KPC],          # lhsT [c=128, k'=128]
            a_t[:, wa + n * B : wa + (n + 1) * B],    # rhs  [c=128, b=32]
            start=(n == 0),
            stop=False,
        )
    nc.tensor.wait_ge(s_b, 16)
    for n in range(NB):
        mm = nc.tensor.matmul(
            ps[:],
            b_t[:, n * KPC : (n + 1) * KPC],
            b_t[:, wb + n * B : wb + (n + 1) * B],
            start=False,
            stop=(n == NB - 1),
        )
    mm.then_inc(s_pe, 1)

    # DVE: PSUM + per-partition bias -> SBUF in one op.
    nc.vector.wait_ge(s_pe, 1)
    nc.vector.wait_ge(s_bias, 16)
    nc.vector.tensor_scalar_add(o_t[:], ps[:], bi_t[:]).then_inc(s_dve, 1)

    # SP: store, then reset the sems while the store is in flight. No
    # completion WAIT on the store (see module docstring); s_out still gets
    # the mandatory completion increments, but nothing ever waits on it, so
    # clearing it early (possibly racing the increments) is harmless.
    nc.sync.wait_ge(s_dve, 1)
    nc.sync.dma_start(out[:], o_t[:]).then_inc(s_out, 16)
    nums = sorted(s.num for s in sems)
    if nums == list(range(nums[0], nums[0] + len(nums))):
        nc.sync.sem_clear(range(nums[0], nums[-1] + 1))
    else:  # non-contiguous allocation — clear individually
        for s in sems:
            nc.sync.sem_clear(s)

    nc.finalize()
    return nc


def kernel(x: np.ndarray, weights: np.ndarray, bias: np.ndarray) -> np.ndarray:
    global _NC, LAST_RESULT
    if _NC is None:
        _NC = _build_nc()

    x = np.asarray(x, dtype=np.float32)
    weights = np.asarray(weights, dtype=np.float32)
    bias = np.asarray(bias, dtype=np.float32)

    # Effective dense weight block and bias (see module docstring).
    w_eff = weights[NODES - OUT_F :, :IN_F][::-1]  # [1024 (k), 1024 (c)]
    b_eff = bias[NODES - OUT_F :][::-1]            # [1024]

    # Pack per-core ring operands. w_eff[(i,k'),(n,p)] -> wt[i][p, (n,k')]
    wt_all = w_eff.reshape(N_CORES, KPC, NCHUNK, 128).transpose(0, 3, 2, 1)
    wt_all = wt_all.reshape(N_CORES, 128, NCHUNK, KPC).astype(ml_dtypes.bfloat16)
    # x[b, (n,p)] -> xt[p, (n,b)], replicated
    xt = (
        x.reshape(B, NCHUNK, 128).transpose(2, 1, 0).astype(ml_dtypes.bfloat16)
    )  # [p, n, b]
    b_all = np.ascontiguousarray(b_eff.reshape(N_CORES, KPC, 1))

    in_maps = []
    for i in range(N_CORES):
        ina = np.concatenate(
            [
                wt_all[i, :, :NA].reshape(128, NA * KPC),
                xt[:, :NA].reshape(128, NA * B),
            ],
            axis=1,
        )
        inb = np.concatenate(
            [
                wt_all[i, :, NA:].reshape(128, NB * KPC),
                xt[:, NA:].reshape(128, NB * B),
            ],
            axis=1,
        )
        in_maps.append(
            {
                "ina": np.ascontiguousarray(ina),
                "inb": np.ascontiguousarray(inb),
                "bi": b_all[i],
            }
        )
    LAST_RESULT = run_bass_kernel_spmd(_NC, in_maps, list(range(N_CORES)))

    # Gather: core i returns out[k', b] for k = i*KPC + k'.
    out_t = np.concatenate([r["out"] for r in LAST_RESULT.results], axis=0)
    return np.ascontiguousarray(out_t.T)


# revision 22
# speedup vs baseline: 1.1267x; 1.0245x over previous
"""Trainium2 Bass kernel for the gnn_message_passing problem.

Math reduction: the reference builds a [8192,8192] zero-diagonal adjacency
W_full from per-node Linear(8191,1) weights, forms state = [x | zeros] and
returns (state @ W_full.T + bias)[:, 7168:][:, ::-1].

Because state is zero outside its first 1024 columns, and only output nodes
7168..8191 are read, the whole computation collapses to

    out[b, k] = sum_c x[b, c] * weights[8191-k, c] + bias[8191-k]

i.e. a [32,1024] x [1024,1024]^T matmul + bias (for rows n >= 7168 and
cols c < 1024 we always have c < n, so W_full[n, c] == weights[n, c]).

Distribution: shard the 1024 output features row-wise across 8 cores
(128 each, tensor parallel); every core holds the replicated x. No
collectives — the host concatenates the 8 output slices.

Per-core kernel (raw bacc, hand-rolled semaphores — no TileContext, whose
drain/barrier/sem-clear tail costs ~2us): weights and x are cast to bf16 on
host (measured rel err ~2.7e-3 vs the f32 reference, well under the 2e-2
gate) and packed into TWO [128, 640] dram tensors, each carrying 4
contraction chunks of W plus the matching 4 chunks of x, so each HWDGE ring
(SP, ACT) moves one big DMA with 1.25KB-per-partition descriptors; the f32
bias trails ring A, gated by its own semaphore so it cannot delay the
weights. Eight PSUM-accumulated bf16 matmuls run as ring data lands; the
epilogue is a single DVE tensor_scalar add (PSUM + bias -> SBUF — no PE
bias-matmul in the accumulation tail and no activation-table load that
would stall the ACT HWDGE ring). SP issues the output DMA with its
mandatory completion increments but does NOT wait on them — the runtime's
end-of-execution epilogue drains the DMA queues, so the in-flight store
overlaps the fixed ~7us teardown sweep (validated by the --warm rerun in
test.py) — then resets the semaphores in one range clear so repeated NEFF
executions stay correct.
"""

import numpy as np
import ml_dtypes

import concourse.bacc as bacc
import concourse.mybir as mybir
from concourse.bass_utils import run_bass_kernel_spmd

NODES = 8192
IN_F = 1024
OUT_F = 1024
B = 32
N_CORES = 8
KPC = OUT_F // N_CORES   # output features per core: 128
NCHUNK = IN_F // 128     # contraction chunks: 8
# Ring A (SP HWDGE) streams ~115 GB/s but starts ~0.5us before ring B
# (ACT HWDGE, ~199 GB/s) — 3 chunks on A and 5 on B makes both streams
# finish together (measured: equal-split left ring A ~0.4us behind).
NA = 3                   # contraction chunks on ring A
NB = NCHUNK - NA         # contraction chunks on ring B: 5
CPB = KPC + B            # cols per chunk (weights + x block): 160

F32 = mybir.dt.float32
BF16 = mybir.dt.bfloat16

_NC = None
LAST_RESULT = None  # BassKernelResults of the most recent run (for profiling)


def _build_nc():
    nc = bacc.Bacc(None, target_bir_lowering=False)

    # Ring tensors, packed on host (bf16); ring X carries chunks [off, off+S):
    #   inX[p, n*KPC + k'] = W_eff[core*KPC + k', (n + off)*128 + p]  n<S
    #   inX[p, S*KPC + n*B + b] = x[b, (n + off)*128 + p]
    ina = nc.dram_tensor("ina", [128, NA * CPB], BF16, kind="ExternalInput")
    inb = nc.dram_tensor("inb", [128, NB * CPB], BF16, kind="ExternalInput")
    bi = nc.dram_tensor("bi", [KPC, 1], F32, kind="ExternalInput")
    out = nc.dram_tensor("out", [KPC, B], F32, kind="ExternalOutput")

    a_t = nc.alloc_sbuf_tensor("a_t", [128, NA * CPB], BF16)
    b_t = nc.alloc_sbuf_tensor("b_t", [128, NB * CPB], BF16)
    bi_t = nc.alloc_sbuf_tensor("bi_t", [KPC, 1], F32)
    o_t = nc.alloc_sbuf_tensor("o_t", [KPC, B], F32)
    ps = nc.alloc_psum_tensor("ps", [KPC, B], F32)

    s_a = nc.alloc_semaphore("s_a")
    s_b = nc.alloc_semaphore("s_b")
    s_bias = nc.alloc_semaphore("s_bias")
    s_pe = nc.alloc_semaphore("s_pe")
    s_dve = nc.alloc_semaphore("s_dve")
    s_out = nc.alloc_semaphore("s_out")
    sems = (s_a, s_b, s_bias, s_pe, s_dve, s_out)

    # Ring warm-up: a 1-descriptor dummy DMA on each HWDGE ring pays the
    # ~1.2us doorbell->SDMA-startup latency while the big transfers'
    # descriptors are still being generated. The increments land on s_out,
    # which nothing ever waits on.
    wu_a = nc.alloc_sbuf_tensor("wu_a", [1, 256], BF16)
    wu_b = nc.alloc_sbuf_tensor("wu_b", [1, 256], BF16)
    nc.sync.dma_start(wu_a[0:1, :], ina[0:1, 0:256]).then_inc(s_out, 16)
    nc.scalar.dma_start(wu_b[0:1, :], inb[0:1, 0:256]).then_inc(s_out, 16)

    # One big DMA per HWDGE ring: W/x chunks [0,NA) on SP, [NA,8) on ACT
    # (ACT does nothing else — no act-table load ahead on its ring). The f32
    # bias trails ring A: its 128 thin descriptors run after inA's fat ones,
    # so they cannot delay s_a, and only the DVE epilogue waits on s_bias.
    nc.sync.dma_start(a_t[:], ina[:]).then_inc(s_a, 16)
    nc.sync.dma_start(bi_t[:], bi[:]).then_inc(s_bias, 16)
    nc.scalar.dma_start(b_t[:], inb[:]).then_inc(s_b, 16)

    # PE: 8 PSUM-accumulated matmuls, ring A's chunks then ring B's.
    wa, wb = NA * KPC, NB * KPC
    nc.tensor.wait_ge(s_a, 16)
    for n in range(NA):
        nc.tensor.matmul(
            ps[:],
            a_t[:, n * KPC : (n + 1) * KPC],          # lhsT [c=128, k'=128]
            a_t[:, wa + n * B : wa + (n + 1) * B],    # rhs  [c=128, b=32]
            start=(n == 0),
            stop=False,
        )
    nc.tensor.wait_ge(s_b, 16)
    for n in range(NB):
        mm = nc.tensor.matmul(
            ps[:],
            b_t[:, n * KPC : (n + 1) * KPC],
            b_t[:, wb + n * B : wb + (n + 1) * B],
            start=False,
            stop=(n == NB - 1),
        )
    mm.then_inc(s_pe, 1)

    # DVE: PSUM + per-partition bias -> SBUF in one op.
    nc.vector.wait_ge(s_pe, 1)
    nc.vector.wait_ge(s_bias, 16)
    nc.vector.tensor_scalar_add(o_t[:], ps[:], bi_t[:]).then_inc(s_dve, 1)

    # SP: store, then reset the sems while the store is in flight. No
    # completion WAIT on the store (see module docstring); s_out still gets
    # the mandatory completion increments, but nothing ever waits on it, so
    # clearing it early (possibly racing the increments) is harmless.
    nc.sync.wait_ge(s_dve, 1)
    nc.sync.dma_start(out[:], o_t[:]).then_inc(s_out, 16)
    nums = sorted(s.num for s in sems)
    if nums == list(range(nums[0], nums[0] + len(nums))):
        nc.sync.sem_clear(range(nums[0], nums[-1] + 1))
    else:  # non-contiguous allocation — clear individually
        for s in sems:
            nc.sync.sem_clear(s)

    nc.finalize()
    return nc


def kernel(x: np.ndarray, weights: np.ndarray, bias: np.ndarray) -> np.ndarray:
    global _NC, LAST_RESULT
    if _NC is None:
        _NC = _build_nc()

    x = np.asarray(x, dtype=np.float32)
    weights = np.asarray(weights, dtype=np.float32)
    bias = np.asarray(bias, dtype=np.float32)

    # Effective dense weight block and bias (see module docstring).
    w_eff = weights[NODES - OUT_F :, :IN_F][::-1]  # [1024 (k), 1024 (c)]
    b_eff = bias[NODES - OUT_F :][::-1]            # [1024]

    # Pack per-core ring operands. w_eff[(i,k'),(n,p)] -> wt[i][p, (n,k')]
    wt_all = w_eff.reshape(N_CORES, KPC, NCHUNK, 128).transpose(0, 3, 2, 1)
    wt_all = wt_all.reshape(N_CORES, 128, NCHUNK, KPC).astype(ml_dtypes.bfloat16)
    # x[b, (n,p)] -> xt[p, (n,b)], replicated
    xt = (
        x.reshape(B, NCHUNK, 128).transpose(2, 1, 0).astype(ml_dtypes.bfloat16)
    )  # [p, n, b]
    b_all = np.ascontiguousarray(b_eff.reshape(N_CORES, KPC, 1))

    in_maps = []
    for i in range(N_CORES):
        ina = np.concatenate(
            [
                wt_all[i, :, :NA].reshape(128, NA * KPC),
                xt[:, :NA].reshape(128, NA * B),
            ],
            axis=1,
        )
        inb = np.concatenate(
            [
                wt_all[i, :, NA:].reshape(128, NB * KPC),
                xt[:, NA:].reshape(128, NB * B),
            ],
            axis=1,
        )
        in_maps.append(
            {
                "ina": np.ascontiguousarray(ina),
                "inb": np.ascontiguousarray(inb),
                "bi": b_all[i],
            }
        )
    LAST_RESULT = run_bass_kernel_spmd(_NC, in_maps, list(range(N_CORES)))

    # Gather: core i returns out[k', b] for k = i*KPC + k'.
    out_t = np.concatenate([r["out"] for r in LAST_RESULT.results], axis=0)
    return np.ascontiguousarray(out_t.T)
